# revision 26
# baseline (speedup 1.0000x reference)
"""Trainium2 Bass kernel for nn_AssociationScore (GCN + MLP scoring head).

The computation is linear up to the final sigmoid, so the 64-dim GCN
aggregation collapses to a per-node scalar:
    w3  = W @ w2                       (256-vector, computed on device)
    u   = x @ w3                       (per-node scalar; memory-bound matvec)
    g   = u * dinv                     (dinv = rsqrt(indeg + 1))
    z0[d] = sum over edges (s->d) of g[s]
    score = sigmoid(dinv*z0 + dinv^2*u + (b@w2 + b2))

Sharding: nodes row-sharded over 8 NeuronCores (12500/core). Each core
computes its u/g shard, all-gathers g, then aggregates the edges whose dst
lies in its shard. The scatter-sum runs as: per-source-octant GPSIMD
ap_gather of g (8 independent index streams, one per Q7 core) -> DVE prefix
scan along the free dim -> second ap_gather at segment-boundary positions ->
differences. Host-side work is integer routing only (sort + CSR offsets +
int16 index tables); all floating-point math runs on device.

Matvec: stationary = w3 broadcast across PE columns, moving = x chunks, so
the result u lands replicated across partitions in PSUM; per-chunk rows are
copied to a [28, 448] SBUF tile and bounced through DRAM to reach the
[98, 128] layout used by the aggregation tail.
"""
import numpy as np

NCORES = 8
N = 100000
M = 12500            # nodes per core
TILES = 98           # 98 * 128 = 12544
MP = TILES * 128
D = 256
H = 64
NCH = 16             # gather/scan pipeline chunks
NBCH = 1             # boundary gather chunks
MVC = 28             # matvec chunks
MVF = 448            # matvec chunk columns (28*448 = 12544)


def _routing(src, dst):
    """Integer routing tables. Returns per-core wrapped int16 index arrays and sizes."""
    core = dst // M
    octv = src // M
    key = (core * 8 + octv) * np.int64(N) + dst
    order = np.argsort(key, kind='stable')
    s_s, d_s = src[order], dst[order]
    grp = (core * 8 + octv)[order]
    counts = np.bincount(grp, minlength=64)
    KE = int(counts.max()) + 1                      # +1 sentinel at stream head
    KE = ((KE + NCH * 64 - 1) // (NCH * 64)) * (NCH * 64)
    assert KE <= 32768
    offs = np.concatenate([[0], np.cumsum(counts)])
    idx_main = np.full((NCORES, 8, KE), M, np.int16)    # sentinel idx = M (zero pad row)
    ends = np.zeros((NCORES, 8, MP), np.int64)
    for c in range(NCORES):
        for o in range(8):
            gi = c * 8 + o
            L = counts[gi]
            sl = slice(offs[gi], offs[gi] + L)
            idx_main[c, o, 1:1 + L] = (s_s[sl] - o * M).astype(np.int16)
            dl = d_s[sl] - c * M
            cnt = np.bincount(dl, minlength=M)
            e = np.cumsum(cnt)                      # stream pos of last edge with dst<=n
            ends[c, o, :M] = e
            ends[c, o, M:] = e[-1]
    EKE = ((MP + TILES + NBCH * 64 - 1) // (NBCH * 64)) * (NBCH * 64)
    ext = np.zeros((NCORES, 8, EKE), np.int64)
    ext[:, :, :MP] = ends
    for j in range(1, TILES):
        ext[:, :, MP + j] = ends[:, :, 128 * j - 1]
    deg = (np.bincount(dst, minlength=N) + 1).astype(np.float32)

    def wrap16(a):                                  # [8, K] -> [128, K//16]
        o, K = a.shape
        return np.ascontiguousarray(
            a.reshape(o, K // 16, 16).transpose(0, 2, 1).reshape(o * 16, K // 16))

    idxm = np.stack([wrap16(idx_main[c]) for c in range(NCORES)])
    idxe = np.stack([wrap16(ext[c].astype(np.int16)) for c in range(NCORES)])
    return idxm, idxe, deg, KE, EKE


def _emit(nc, tc, t, KE, EKE, variant=9):
    """Emit the per-core program. `t` maps tensor names to DRAM APs."""
    import concourse.mybir as mybir
    import concourse.tile as tile  # noqa: F401

    dt = mybir.dt
    f32 = dt.float32
    Alu = mybir.AluOpType

    with tc.tile_pool(name="const", bufs=1) as cpool, \
         tc.tile_pool(name="big", bufs=1) as bpool, \
         tc.tile_pool(name="scr", bufs=1) as spool, \
         tc.tile_pool(name="gchp", bufs=2) as gpool, \
         tc.tile_pool(name="gchs", bufs=1) as gspool, \
         tc.tile_pool(name="ps", bufs=1, space="PSUM") as ppool, \
         tc.tile_pool(name="psp", bufs=2, space="PSUM") as prodpool, \
         tc.tile_pool(name="psg", bufs=2, space="PSUM") as gbpool, \
         tc.tile_pool(name="dram", bufs=1, space="DRAM") as dpool:

        # ---- constants
        wTt = cpool.tile([H, D], f32)
        nc.sync.dma_start(wTt[:], t["wT"])
        w2rt = cpool.tile([H, 128], f32)
        nc.sync.dma_start(w2rt[:], t["w2r"])
        bcolt = cpool.tile([H, 1], f32)
        nc.sync.dma_start(bcolt[:], t["bcol"])
        b2rt = cpool.tile([128, 1], f32)
        nc.sync.dma_start(b2rt[:], t["b2r"])
        idxmt = cpool.tile([128, KE // 16], dt.int16)
        nc.scalar.dma_start(idxmt[:], t["idxm"])
        idxet = cpool.tile([128, EKE // 16], dt.int16)
        nc.scalar.dma_start(idxet[:], t["idxe"])
        degt98 = cpool.tile([TILES, 128], f32)
        nc.sync.dma_start(degt98[:], t["deg"].rearrange("(a b) -> a b", a=TILES))
        obt = cpool.tile([NCORES, 128], dt.bfloat16)
        nc.sync.dma_start(obt[:], t["ob"])

        # ---- w3 halves [128, 1] -> bf16 -> broadcast [128, 128]
        w3c0ps = ppool.tile([128, 1], f32, tag="w3c0")
        nc.tensor.matmul(w3c0ps[:], wTt[:, 0:128], w2rt[:, 0:1],
                         start=True, stop=True)
        w3c1ps = ppool.tile([128, 1], f32, tag="w3c1")
        nc.tensor.matmul(w3c1ps[:], wTt[:, 128:256], w2rt[:, 0:1],
                         start=True, stop=True)
        w3c0 = cpool.tile([128, 1], dt.bfloat16)
        nc.vector.tensor_copy(w3c0[:], w3c0ps[:])
        w3c1 = cpool.tile([128, 1], dt.bfloat16)
        nc.vector.tensor_copy(w3c1[:], w3c1ps[:])
        w3rep0 = cpool.tile([128, 128], dt.bfloat16)
        nc.vector.tensor_copy(w3rep0[:], w3c0[:].broadcast_to([128, 128]))
        w3rep1 = cpool.tile([128, 128], dt.bfloat16)
        nc.vector.tensor_copy(w3rep1[:], w3c1[:].broadcast_to([128, 128]))

        # ---- c0 = b@w2 + b2, replicated [128, 1]
        c0ps = ppool.tile([128, 1], f32, tag="c0ps")
        nc.tensor.matmul(c0ps[:], w2rt[:], bcolt[:], start=True, stop=True)
        c0t = cpool.tile([128, 1], f32)
        nc.vector.tensor_add(c0t[:], c0ps[:], b2rt[:])

        # ---- matvec u = x @ w3: stationary = w3 broadcast, moving = x chunk.
        # Each PSUM chunk holds u for 448 nodes replicated on all partitions;
        # row c of chunk c is DMA'd to the node-order Ushard in DRAM.
        ushard = dpool.tile([1, MP], f32)
        BF = 4 * MVF                     # 1792 cols per load/store group
        NBG = MP // BF                   # 7 groups
        usv = ushard.opt()[0].rearrange("(c f) -> c f", c=NBG)
        xsv = t["xs"].rearrange("(h k) (c f) -> h c k f", k=128, f=BF)
        with tc.tile_pool(name="xload", bufs=2) as xpool, \
             tc.tile_pool(name="ucp", bufs=2) as upool:
            for c in range(NBG):
                eng = nc.sync if c % 2 == 0 else nc.scalar
                xa = xpool.tile([128, BF], dt.bfloat16, tag="xa")
                eng.dma_start(xa[:], xsv[0][c])
                xb = xpool.tile([128, BF], dt.bfloat16, tag="xb")
                eng.dma_start(xb[:], xsv[1][c])
                usb = upool.tile([128, BF], f32, tag="usb")
                for a in range(4):
                    sl = slice(a * MVF, (a + 1) * MVF)
                    ps = prodpool.tile([128, MVF], f32, tag="mv")
                    nc.tensor.matmul(ps[:], w3rep0[:], xa[:, sl],
                                     start=True, stop=False)
                    nc.tensor.matmul(ps[:], w3rep1[:], xb[:, sl],
                                     start=False, stop=True)
                    if a % 2 == 0:
                        nc.vector.tensor_copy(usb[:, sl], ps[:])
                    else:
                        nc.scalar.copy(usb[:, sl], ps[:])
                eng2 = nc.scalar if c % 2 == 0 else nc.sync
                eng2.dma_start(usv[c:c + 1], usb[c:c + 1, :])

        # ---- dinv [98, 128]; u readback; g = dinv*u; gshard (node order)
        rec98 = spool.tile([TILES, 128], f32, tag="rec98")
        nc.vector.reciprocal(rec98[:], degt98[:])
        dinvt = bpool.tile([TILES, 128], f32)
        nc.scalar.sqrt(dinvt[:], rec98[:])
        ublk = spool.tile([TILES, 128], f32, tag="ublk")
        nc.sync.dma_start(
            ublk[:], ushard.opt()[0].rearrange("(a b) -> a b", a=TILES))
        gblk = bpool.tile([TILES, 128], f32)
        nc.vector.tensor_mul(gblk[:], dinvt[:], ublk[:])
        gbf = spool.tile([TILES, 128], dt.bfloat16, tag="gbf")
        nc.vector.tensor_copy(gbf[:], gblk[:])
        gshard = dpool.tile([1, MP], dt.bfloat16)
        nc.sync.dma_start(
            gshard.opt()[0].rearrange("(a b) -> a b", a=TILES), gbf[:])

        if variant <= 4:
            res0 = spool.tile([TILES, 128], f32, tag="res")
            nc.vector.tensor_copy(res0[:], gblk[:])
            nc.sync.dma_start(t["out"].rearrange("(a b) -> a b", a=TILES), res0[:])
            return

        # ---- allgather g (bf16 shards)
        gfull = dpool.tile([NCORES, MP], dt.bfloat16)
        nc.gpsimd.collective_compute(
            "AllGather", Alu.bypass,
            replica_groups=[list(range(NCORES))],
            ins=[gshard.opt()], outs=[gfull.opt()])
        if variant == 56:      # 7 extra serialized allgathers to measure CC cost
            for _ in range(7):
                nc.gpsimd.collective_compute(
                    "AllGather", Alu.bypass,
                    replica_groups=[list(range(NCORES))],
                    ins=[gshard.opt()], outs=[gfull.opt()])

        if variant in (55, 56):
            res1 = spool.tile([TILES, 128], f32, tag="res")
            nc.vector.tensor_copy(res1[:], gblk[:])
            nc.sync.dma_start(t["out"].rearrange("(a b) -> a b", a=TILES), res1[:])
            return

        # ---- octant table [128, MP] f32: PE replicates the 8 shards into
        # partition groups via a one-hot stationary (gtab[16o+r, :] = shard o).
        gtab = bpool.tile([128, MP], f32, tag="gtb")
        GF = 7 * MVF                     # 3136 cols per gfull load
        with tc.tile_pool(name="gfl", bufs=2) as gfpool:
            for c in range(MP // GF):
                gfsb = gfpool.tile([NCORES, GF], dt.bfloat16, tag="gf")
                eng = nc.sync if c % 2 == 0 else nc.scalar
                eng.dma_start(gfsb[:], gfull.opt()[:, c * GF:(c + 1) * GF])
                for a in range(7):
                    sl = slice(c * GF + a * MVF, c * GF + (a + 1) * MVF)
                    pst = gbpool.tile([128, MVF], f32, tag="gb")
                    nc.tensor.matmul(pst[:], obt[:], gfsb[:, a * MVF:(a + 1) * MVF],
                                     start=True, stop=True)
                    if a % 2 == 0:
                        nc.vector.tensor_copy(gtab[:, sl], pst[:])
                    else:
                        nc.scalar.copy(gtab[:, sl], pst[:])

        if variant <= 6:
            res2 = spool.tile([TILES, 128], f32, tag="res")
            nc.vector.tensor_copy(res2[:], gtab[0:TILES, 0:128])
            nc.sync.dma_start(t["out"].rearrange("(a b) -> a b", a=TILES), res2[:])
            return

        # ---- gather + prefix scan over edge streams
        scn = bpool.tile([128, KE], f32)
        ones1 = cpool.tile([128, 1], f32)
        nc.vector.memset(ones1[:], 1.0)
        CH = KE // NCH
        gtab3 = gtab[:].rearrange("p (n d) -> p n d", d=1)
        for tt in range(NCH):
            if variant in (81, 82):
                # gather_transpose: byte-granular gather of the f32 table;
                # output is two u16 planes to re-interleave before the scan
                gtr = gpool.tile([128, 4 * CH], dt.uint8, tag="gtr")
                nc.gpsimd.gather_transpose(
                    out_ap=gtr[:].rearrange("p (d n) -> p d n", d=4),
                    in_ap=gtab[:].bitcast(dt.uint8).rearrange(
                        "p (n d) -> p n d", d=4),
                    idxs_ap=idxmt[:, tt * (CH // 16):(tt + 1) * (CH // 16)],
                    channels=128, num_elems=MP, d=4,
                    num_idxs=CH)
                gch = gspool.tile([128, CH], f32, tag="gch")
                gu16 = gtr[:].bitcast(dt.uint16).rearrange(
                    "p (d n) -> p d n", d=2)
                if variant == 81:   # HW: 16-bit-granular planes
                    out2 = gch[:].bitcast(dt.uint16).rearrange(
                        "p (n k) -> p k n", k=2)
                    nc.vector.tensor_copy(out2[:, 0, :], gu16[:, 0, :])
                    nc.vector.tensor_copy(out2[:, 1, :], gu16[:, 1, :])
                else:               # sim/interp: byte-granular planes
                    gu8 = gtr[:].rearrange("p (d n) -> p d n", d=4)
                    out4 = gch[:].bitcast(dt.uint8).rearrange(
                        "p (n k) -> p k n", k=4)
                    for bb in range(4):
                        nc.vector.tensor_copy(out4[:, bb, :], gu8[:, bb, :])
            else:
                gch = gpool.tile([128, CH], f32, tag="gch")
                nc.gpsimd.ap_gather(
                    out_ap=gch[:].rearrange("p (n d) -> p n d", d=1),
                    in_ap=gtab3,
                    idxs_ap=idxmt[:, tt * (CH // 16):(tt + 1) * (CH // 16)],
                    channels=128, num_elems=MP, d=1,
                    num_idxs=CH)
            init = 0.0 if tt == 0 else scn[:, tt * CH - 1:tt * CH]
            nc.vector.tensor_tensor_scan(
                out=scn[:, tt * CH:(tt + 1) * CH],
                data0=ones1[:].broadcast_to([128, CH]),
                data1=gch[:],
                initial=init, op0=Alu.mult, op1=Alu.add)

        if variant <= 8 or variant == 83:
            res4 = spool.tile([TILES, 128], f32, tag="res")
            nc.vector.tensor_copy(res4[:], scn[0:TILES, 0:128])
            nc.sync.dma_start(t["out"].rearrange("(a b) -> a b", a=TILES), res4[:])
            return

        # ---- boundary extraction, chunked; each chunk's octant rows are
        # bounced to DRAM right away so the writes hide under later chunks
        bexf = bpool.tile([128, max(MP, EKE)], f32, tag="gtb")
        bex = bexf[:, 0:EKE]
        bexd = dpool.tile([8, EKE], f32)
        BCH = EKE // NBCH
        scn3 = scn[:].rearrange("p (n d) -> p n d", d=1)
        for bt in range(NBCH):
            sl = slice(bt * BCH, (bt + 1) * BCH)
            nc.gpsimd.ap_gather(
                out_ap=bex[:, sl].rearrange("p (n d) -> p n d", d=1),
                in_ap=scn3,
                idxs_ap=idxet[:, bt * (BCH // 16):(bt + 1) * (BCH // 16)],
                channels=128, num_elems=KE, d=1,
                num_idxs=BCH)
            eng = nc.sync if bt % 2 == 0 else nc.scalar
            eng.dma_start(bexd.opt()[:, sl], bexf[0:128:16, sl])

        # ---- redistribute per octant to block layout via DRAM bounce
        Ball = bpool.tile([TILES, 8 * 128], f32)
        Bx = bpool.tile([TILES, 8], f32)
        nc.sync.dma_start(
            Ball[:].rearrange("j (o c) -> j o c", o=8),
            bexd.opt()[:, 0:MP].rearrange("o (j c) -> j o c", j=TILES))
        nc.scalar.dma_start(
            Bx[:],
            bexd.opt()[:, MP:MP + TILES].rearrange("o j -> j o"))

        # ---- merge octants, diff, combine, sigmoid
        Zt = spool.tile([TILES, 128], f32, tag="zt")
        nc.vector.tensor_add(Zt[:], Ball[:, 0:128], Ball[:, 128:256])
        for o in range(2, 8):
            nc.vector.tensor_add(Zt[:], Zt[:], Ball[:, o * 128:(o + 1) * 128])
        Zx = spool.tile([TILES, 1], f32, tag="zx")
        nc.vector.tensor_reduce(Zx[:], Bx[:], axis=mybir.AxisListType.X,
                                op=Alu.add)
        Dt = spool.tile([TILES, 128], f32, tag="dt")
        nc.vector.tensor_sub(Dt[:, 1:128], Zt[:, 1:128], Zt[:, 0:127])
        nc.vector.tensor_sub(Dt[:, 0:1], Zt[:, 0:1], Zx[:])
        t1 = spool.tile([TILES, 128], f32, tag="t1")
        nc.vector.tensor_add(t1[:], Dt[:], gblk[:])
        t2 = spool.tile([TILES, 128], f32, tag="t2")
        nc.vector.tensor_mul(t2[:], dinvt[:], t1[:])
        res = spool.tile([TILES, 128], f32, tag="res")
        nc.scalar.activation(res[:], t2[:],
                             mybir.ActivationFunctionType.Sigmoid,
                             bias=c0t[0:TILES, :])
        nc.sync.dma_start(t["out"].rearrange("(a b) -> a b", a=TILES), res[:])


def _build_nc(KE, EKE, variant=None):
    import os
    if variant is None:
        variant = int(os.environ.get("KVAR", "9"))
    import concourse.bacc as bacc
    import concourse.mybir as mybir
    import concourse.tile as tile

    dt = mybir.dt
    f32 = dt.float32
    nc = bacc.Bacc("TRN2", target_bir_lowering=False, debug=False,
                   num_devices=NCORES)
    t = {
        "xs": nc.dram_tensor("xs", [D, MP], dt.bfloat16, kind="ExternalInput").ap(),
        "wT": nc.dram_tensor("wT", [H, D], f32, kind="ExternalInput").ap(),
        "w2r": nc.dram_tensor("w2r", [H, 128], f32, kind="ExternalInput").ap(),
        "bcol": nc.dram_tensor("bcol", [H, 1], f32, kind="ExternalInput").ap(),
        "b2r": nc.dram_tensor("b2r", [128, 1], f32, kind="ExternalInput").ap(),
        "deg": nc.dram_tensor("deg", [MP], f32, kind="ExternalInput").ap(),
        "ob": nc.dram_tensor("ob", [NCORES, 128], dt.bfloat16, kind="ExternalInput").ap(),
        "idxm": nc.dram_tensor("idxm", [128, KE // 16], dt.int16, kind="ExternalInput").ap(),
        "idxe": nc.dram_tensor("idxe", [128, EKE // 16], dt.int16, kind="ExternalInput").ap(),
        "out": nc.dram_tensor("out", [MP], f32, kind="ExternalOutput").ap(),
    }
    with tile.TileContext(nc) as tc:
        _emit(nc, tc, t, KE, EKE, variant)
    nc.compile()
    return nc


def _make_in_maps(x, edge_index, W, b, w2, b2):
    src = np.asarray(edge_index[0], dtype=np.int64)
    dst = np.asarray(edge_index[1], dtype=np.int64)
    idxm, idxe, deg, KE, EKE = _routing(src, dst)

    xf = np.asarray(x, dtype=np.float32)
    Wf = np.asarray(W, dtype=np.float32)
    w2f = np.asarray(w2, dtype=np.float32).reshape(H)
    bf = np.asarray(b, dtype=np.float32)
    b2f = np.asarray(b2, dtype=np.float32).reshape(1)

    wT = np.ascontiguousarray(Wf.T)                       # [H, D]
    w2rep = np.ascontiguousarray(np.broadcast_to(w2f.reshape(H, 1), (H, 128)))
    bcol = bf.reshape(H, 1)
    b2rep = np.full((128, 1), float(b2f[0]), np.float32)

    import ml_dtypes
    ob = np.zeros((NCORES, 128), np.float32)
    for o in range(NCORES):
        ob[o, o * 16:(o + 1) * 16] = 1.0
    ob = ob.astype(ml_dtypes.bfloat16)
    in_maps = []
    for c in range(NCORES):
        xsp = np.zeros((MP, D), np.float32)
        xsp[:M] = xf[c * M:(c + 1) * M]
        xsp = np.ascontiguousarray(xsp.T).astype(ml_dtypes.bfloat16)
        degp = np.ones(MP, np.float32)
        degp[:M] = deg[c * M:(c + 1) * M]
        in_maps.append({
            "xs": xsp,
            "wT": wT,
            "w2r": w2rep,
            "bcol": bcol,
            "b2r": b2rep,
            "deg": degp,
            "ob": ob,
            "idxm": idxm[c],
            "idxe": idxe[c],
        })
    return in_maps, KE, EKE


def _unshard(outv):
    return np.concatenate([outv[c][:M] for c in range(NCORES)]).astype(np.float32)


def kernel(x, edge_index, W, b, w2, b2):
    in_maps, KE, EKE = _make_in_maps(x, edge_index, W, b, w2, b2)
    nc = _build_nc(KE, EKE)
    from concourse.bass_utils import run_bass_kernel_spmd
    res = None
    for attempt in range(3):
        try:
            res = run_bass_kernel_spmd(nc, in_maps, list(range(NCORES)))
            break
        except Exception:
            if attempt == 2:
                raise
            # device may be in a bad state from a prior run; exercise the
            # plain XLA path on every core to recover, then retry
            import jax
            import jax.numpy as jnp
            a = np.eye(128, dtype=np.float32)
            for d in jax.devices()[:NCORES]:
                jnp.dot(jax.device_put(a, d), jax.device_put(a, d)).block_until_ready()
    outv = np.stack([res.results[c]["out"] for c in range(NCORES)])
    return _unshard(outv)
